# revision 15
# baseline (speedup 1.0000x reference)
"""Causal self-attention (B=1, S=4096, D=1024, 16 heads) on 8 trn2 NeuronCores.

Sharding: tensor-parallel over heads (2 heads per core). Each core computes
qkv projection for its head pair, causal attention, and a partial output
projection; the host sums the 8 partials and adds b_out.

Device kernel (per core, all matmuls in float32r, fp32 PSUM accumulation):
  phase 1 (per 512-seq chunk, pipelined with phase 2): qT/kT/vT =
      w_shard.T @ xT; bias added on the DVE copy out of PSUM. Head B's q/k
      rows are relocated to partitions 0-63 via SBUF->SBUF DMA (matmul
      operands must start at partition 0 on this compiler). V (natural
      layout) is built by PE-transposing vT k-tiles.
  phase 2 (per 512-wide q chunk): scores^T = K_tile.T @ Q per head (K=64),
      exp on ACT (scale=1/8 fused; scores are bounded so no max-subtraction
      is needed), causal-mask multiply on diagonal tiles, PV with an
      appended ones-column ([V|1], M=65) so row 64 of the accumulator is
      the softmax denominator, reciprocal -> DMA row to partition 0 ->
      gpsimd partition-broadcast -> normalize; head B's normalized ctx is
      DMA-relocated to partitions 64-127 so the out-projection runs K=128.
      Diagonal k-tiles restrict the streamed column range [128p:512] in
      QK/exp/mask/PV (the rest is fully masked anyway).
"""
import sys

sys.path.insert(0, "/opt/trn_rl_repo")

from contextlib import ExitStack

import numpy as np

import concourse.tile as tile
from concourse import bacc, mybir
from concourse.alu_op_type import AluOpType
from concourse.masks import make_identity
from concourse.bass_utils import run_bass_kernel_spmd

D = 1024
N_CORES = 8
F32 = mybir.dt.float32
F32R = mybir.dt.float32r
AF = mybir.ActivationFunctionType

QC = 512  # q-chunk width
KT = 128  # k-tile width


def build_program(S: int = 4096, repeat: int = 1):
    nqc = S // QC
    nkt = S // KT

    nc = bacc.Bacc(None)
    xT = nc.declare_dram_parameter("xT", [D, S], F32R, isOutput=False)
    w_sh = nc.declare_dram_parameter("w_sh", [D, 384], F32R, isOutput=False)
    b_sh = nc.declare_dram_parameter("b_sh", [384], F32, isOutput=False)
    w_o = nc.declare_dram_parameter("w_o", [128, D], F32R, isOutput=False)
    outp = nc.declare_dram_parameter("outp", [S, D], F32, isOutput=True)

    with tile.TileContext(nc) as tc, ExitStack() as ctx:
        consts = ctx.enter_context(tc.tile_pool(name="consts", bufs=1))
        big = ctx.enter_context(tc.tile_pool(name="big", bufs=1))
        xpool = ctx.enter_context(tc.tile_pool(name="xp", bufs=2))
        vtpool = ctx.enter_context(tc.tile_pool(name="vt", bufs=2))
        stpool = ctx.enter_context(tc.tile_pool(name="st", bufs=2))
        apool = ctx.enter_context(tc.tile_pool(name="at", bufs=3))
        npool = ctx.enter_context(tc.tile_pool(name="nrm", bufs=1))
        opool = ctx.enter_context(tc.tile_pool(name="ot", bufs=2))
        psS = ctx.enter_context(tc.tile_pool(name="psS", bufs=2, space="PSUM"))
        psCA = ctx.enter_context(tc.tile_pool(name="psCA", bufs=1, space="PSUM"))
        psCB = ctx.enter_context(tc.tile_pool(name="psCB", bufs=1, space="PSUM"))
        psO = ctx.enter_context(tc.tile_pool(name="psO", bufs=2, space="PSUM"))

        # ---- constants
        ident_f = consts.tile([128, 128], F32)
        make_identity(nc, ident_f[:])
        ident = consts.tile([128, 128], F32R)
        nc.vector.tensor_copy(ident[:], ident_f[:])

        ones_f = consts.tile([128, 8], F32)
        nc.gpsimd.memset(ones_f[:], 1.0)

        w_sb = consts.tile([128, 8, 384], F32R)
        for m in range(3):
            nc.sync.dma_start(
                w_sb[:, :, m * 128:(m + 1) * 128],
                w_sh.rearrange("(t p) m -> p t m", p=128)[:, :, m * 128:(m + 1) * 128],
            )
        w_o_sb = consts.tile([128, D], F32R)
        nc.sync.dma_start(w_o_sb[:], w_o[:])
        biases = consts.tile([128, 3], F32)
        nc.sync.dma_start(biases[:], b_sh.rearrange("(m p) -> p m", p=128))

        # per-chunk projection tiles (separate tags so attention on chunk c
        # only depends on projections of chunks <= c)
        qk_t = [
            big.tile([64, 2, 2, QC], F32R, tag=f"qk{n}", name=f"qk{n}")
            for n in range(nqc)
        ]
        v_t = [
            big.tile([128, 4, 130], F32R, tag=f"v{n}", name=f"v{n}")
            for n in range(nqc)
        ]
        for n in range(nqc):
            nc.vector.tensor_copy(
                v_t[n][:].rearrange("p t (g c) -> p t g c", g=2)[:, :, :, 64:65],
                ones_f[:].rearrange("p (t g o) -> p t g o", g=2, o=1),
            )

        for _rep in range(repeat):
            def emit_proj(n):
                xts = []
                for half in range(2):
                    xt = xpool.tile([128, 4, QC], F32R)
                    src = xT.rearrange("(t p) s -> p t s", p=128)
                    nc.sync.dma_start(
                        xt[:],
                        src[:, 4 * half:4 * half + 4, n * QC:(n + 1) * QC],
                    )
                    xts.append(xt)
                stage = stpool.tile([128, 2, QC], F32R)
                for m in range(3):
                    ps = psO.tile([128, QC], F32, tag="mm512")
                    for t in range(8):
                        nc.tensor.matmul(
                            ps[:],
                            w_sb[:, t, m * 128:(m + 1) * 128],
                            xts[t // 4][:, t % 4, :],
                            start=(t == 0),
                            stop=(t == 7),
                        )
                    if m < 2:
                        nc.vector.tensor_scalar_add(
                            qk_t[n][:, 0, m, :], ps[0:64, :], biases[0:64, m:m + 1]
                        )
                        nc.vector.tensor_scalar_add(
                            stage[64:128, m, :], ps[64:128, :],
                            biases[64:128, m:m + 1],
                        )
                        if m == 1:
                            nc.sync.dma_start(
                                qk_t[n][:, 1, :, :], stage[64:128, :, :]
                            )
                    else:
                        vt_c = vtpool.tile([128, QC], F32R)
                        nc.vector.tensor_scalar_add(
                            vt_c[:], ps[:], biases[:, 2:3]
                        )
                        tr = psS.tile([128, 4, 128], F32R, tag="sc")
                        for s in range(4):
                            nc.tensor.transpose(
                                tr[:, s, :], vt_c[:, s * 128:(s + 1) * 128], ident[:]
                            )
                        nc.vector.tensor_copy(
                            v_t[n][:].rearrange("p t (g c) -> p t g c", g=2)[:, :, :, 0:64],
                            tr[:].rearrange("p t (g c) -> p t g c", g=2),
                        )

            def emit_jloop(c):
                ctxA = psCA.tile([65, QC], F32, tag="ctxA")
                ctxB = psCB.tile([65, QC], F32, tag="ctxB")
                jmax = 4 * (c + 1)
                for j in range(jmax):
                    p = j - 4 * c
                    off = max(0, p) * KT
                    n_j, s_j = j // 4, j % 4
                    sc = psS.tile([128, 2, QC], F32)
                    for h in range(2):
                        nc.tensor.matmul(
                            sc[:, h, off:],
                            qk_t[n_j][:, h, 1, s_j * KT:(s_j + 1) * KT],
                            qk_t[c][:, h, 0, off:],
                            start=True, stop=True,
                        )
                    at = apool.tile([128, 2, QC], F32R)
                    nc.scalar.activation(
                        at[:, :, off:], sc[:, :, off:], AF.Exp, scale=0.125
                    )
                    if p >= 0:
                        # zero the upper-triangular wedge in place (both heads
                        # in one op; head dim has pattern step 0):
                        # keep iff (off + q_local) - k - 128*p >= 0
                        nc.gpsimd.affine_select(
                            out=at[:, :, off:], in_=at[:, :, off:],
                            pattern=[[0, 2], [1, QC - off]],
                            compare_op=AluOpType.is_ge,
                            fill=0.0, base=off - KT * p, channel_multiplier=-1,
                        )
                    first, last = (j == 0), (j == jmax - 1)
                    nc.tensor.matmul(
                        ctxA[:, off:], v_t[n_j][:, s_j, 0:65], at[:, 0, off:],
                        start=first, stop=last,
                    )
                    nc.tensor.matmul(
                        ctxB[:, off:], v_t[n_j][:, s_j, 65:130], at[:, 1, off:],
                        start=first, stop=last,
                    )
                return ctxA, ctxB
            def emit_norm(c, ctxA, ctxB):
                recip = npool.tile([65, 2, QC], F32, tag="recip")
                nc.vector.reciprocal(recip[64:65, 0, :], ctxA[64:65, :])
                nc.vector.reciprocal(recip[64:65, 1, :], ctxB[64:65, :])
                scr = npool.tile([1, 2, QC], F32, tag="scr")
                nc.sync.dma_start(scr[:], recip[64:65, :, :])
                bc = npool.tile([64, 2, QC], F32, tag="bc")
                nc.gpsimd.partition_broadcast(bc[:], scr[:])
                ctxn = npool.tile([128, QC], F32R, tag="ctxn")
                nc.vector.tensor_mul(ctxn[0:64, :], ctxA[0:64, :], bc[:, 0, :])
                ctxnB = npool.tile([64, QC], F32R, tag="ctxnB")
                nc.vector.tensor_mul(ctxnB[:], ctxB[0:64, :], bc[:, 1, :])
                nc.sync.dma_start(ctxn[64:128, :], ctxnB[:])

                for s in range(4):
                    ot = opool.tile([128, D], F32)
                    for half in range(2):
                        op = psO.tile([128, QC], F32, tag="mm512")
                        nc.tensor.matmul(
                            op[:],
                            ctxn[:, s * 128:(s + 1) * 128],
                            w_o_sb[:, half * QC:(half + 1) * QC],
                            start=True, stop=True,
                        )
                        nc.vector.tensor_copy(
                            ot[:, half * QC:(half + 1) * QC], op[:]
                        )
                    row = c * QC + s * 128
                    nc.gpsimd.dma_start(outp[row:row + 128, :], ot[:])


            emit_proj(0)
            for c in range(nqc):
                _ctx = emit_jloop(c)
                if c + 1 < nqc:
                    emit_proj(c + 1)
                emit_norm(c, *_ctx)
    nc.compile()
    return nc


_PROGRAM_CACHE: dict = {}


def _get_program(S: int):
    if S not in _PROGRAM_CACHE:
        _PROGRAM_CACHE[S] = build_program(S)
    return _PROGRAM_CACHE[S]


def make_in_maps(x, w_qkv, b_qkv, w_out):
    x = np.asarray(x, dtype=np.float32)
    w_qkv = np.asarray(w_qkv, dtype=np.float32)
    b_qkv = np.asarray(b_qkv, dtype=np.float32)
    w_out = np.asarray(w_out, dtype=np.float32)
    S = x.shape[1]
    xT = np.ascontiguousarray(x.reshape(S, D).T)
    in_maps = []
    for c in range(N_CORES):
        lo, hi = 128 * c, 128 * (c + 1)
        w_shard = np.ascontiguousarray(
            np.concatenate(
                [w_qkv[:, lo:hi], w_qkv[:, D + lo:D + hi], w_qkv[:, 2 * D + lo:2 * D + hi]],
                axis=1,
            )
        )
        b_shard = np.concatenate(
            [b_qkv[lo:hi], b_qkv[D + lo:D + hi], b_qkv[2 * D + lo:2 * D + hi]]
        )
        w_o_shard = np.ascontiguousarray(w_out[lo:hi, :])
        in_maps.append(
            {"xT": xT, "w_sh": w_shard, "b_sh": b_shard, "w_o": w_o_shard}
        )
    return in_maps


def kernel(x, w_qkv, b_qkv, w_out, b_out):
    x = np.asarray(x, dtype=np.float32)
    b_out = np.asarray(b_out, dtype=np.float32)
    B, S, _ = x.shape
    in_maps = make_in_maps(x, w_qkv, b_qkv, w_out)
    nc = _get_program(S)
    res = run_bass_kernel_spmd(nc, in_maps, list(range(N_CORES))).results
    out = res[0]["outp"].copy()
    for c in range(1, N_CORES):
        out += res[c]["outp"]
    out += b_out
    return out.reshape(B, S, D)


# revision 26
# speedup vs baseline: 1.7512x; 1.7512x over previous
"""Causal self-attention (B=1, S=4096, D=1024, 16 heads) on 8 trn2 NeuronCores.

Sharding: tensor-parallel over heads (2 heads per core). Each core computes
qkv projection for its head pair, causal attention, and a partial output
projection; the host sums the 8 partials and adds b_out.

Device kernel (per core, all matmuls in float32r, fp32 PSUM accumulation):
  phase 1 (per 512-seq chunk, pipelined with phase 2): qT/kT/vT =
      w_shard.T @ xT; bias added on the DVE copy out of PSUM. Head B's q/k
      rows are relocated to partitions 0-63 via SBUF->SBUF DMA (matmul
      operands must start at partition 0 on this compiler). V (natural
      layout) is built by PE-transposing vT k-tiles.
  phase 2 (per 512-wide q chunk): scores^T = K_tile.T @ Q per head (K=64),
      exp on ACT (scale=1/8 fused; scores are bounded so no max-subtraction
      is needed), causal-mask multiply on diagonal tiles, PV with an
      appended ones-column ([V|1], M=65) so row 64 of the accumulator is
      the softmax denominator, reciprocal -> DMA row to partition 0 ->
      gpsimd partition-broadcast -> normalize; head B's normalized ctx is
      DMA-relocated to partitions 64-127 so the out-projection runs K=128.
      Diagonal k-tiles restrict the streamed column range [128p:512] in
      QK/exp/mask/PV (the rest is fully masked anyway).
"""
import sys

sys.path.insert(0, "/opt/trn_rl_repo")

from contextlib import ExitStack

import numpy as np

import concourse.tile as tile
from concourse import bacc, mybir
from concourse.alu_op_type import AluOpType
from concourse.masks import make_identity
from concourse.bass_utils import run_bass_kernel_spmd

D = 1024
N_CORES = 8
F32 = mybir.dt.float32
F32R = mybir.dt.float32r
AF = mybir.ActivationFunctionType

QC = 512  # q-chunk width
KT = 128  # k-tile width


def build_program(S: int = 4096, repeat: int = 1):
    nqc = S // QC
    nkt = S // KT

    nc = bacc.Bacc(None)
    xT = nc.declare_dram_parameter("xT", [D, S], F32R, isOutput=False)
    w_sh = nc.declare_dram_parameter("w_sh", [D, 384], F32R, isOutput=False)
    b_sh = nc.declare_dram_parameter("b_sh", [384], F32, isOutput=False)
    w_o = nc.declare_dram_parameter("w_o", [128, D], F32R, isOutput=False)
    outp = nc.declare_dram_parameter("outp", [S, D], F32, isOutput=True)

    with tile.TileContext(nc) as tc, ExitStack() as ctx:
        consts = ctx.enter_context(tc.tile_pool(name="consts", bufs=1))
        big = ctx.enter_context(tc.tile_pool(name="big", bufs=1))
        xpool = ctx.enter_context(tc.tile_pool(name="xp", bufs=3))
        vtpool = ctx.enter_context(tc.tile_pool(name="vt", bufs=2))
        stpool = ctx.enter_context(tc.tile_pool(name="st", bufs=3))
        apool = ctx.enter_context(tc.tile_pool(name="at", bufs=4))
        npool = ctx.enter_context(tc.tile_pool(name="nrm", bufs=2))
        opool = ctx.enter_context(tc.tile_pool(name="ot", bufs=3))
        psS = ctx.enter_context(tc.tile_pool(name="psS", bufs=2, space="PSUM"))
        psCA = ctx.enter_context(tc.tile_pool(name="psCA", bufs=1, space="PSUM"))
        psCB = ctx.enter_context(tc.tile_pool(name="psCB", bufs=1, space="PSUM"))
        psO = ctx.enter_context(tc.tile_pool(name="psO", bufs=2, space="PSUM"))

        # ---- constants
        ident_f = consts.tile([128, 128], F32)
        make_identity(nc, ident_f[:])
        ident = consts.tile([128, 128], F32R)
        nc.vector.tensor_copy(ident[:], ident_f[:])

        ones_f = consts.tile([128, 8], F32)
        nc.gpsimd.memset(ones_f[:], 1.0)

        w_sb = consts.tile([128, 8, 384], F32R)
        biases = consts.tile([128, 3], F32)
        nc.sync.dma_start(biases[:], b_sh.rearrange("(m p) -> p m", p=128))
        for m in range(3):
            nc.sync.dma_start(
                w_sb[:, :, m * 128:(m + 1) * 128],
                w_sh.rearrange("(t p) m -> p t m", p=128)[:, :, m * 128:(m + 1) * 128],
            )
        w_o_sb = consts.tile([128, D], F32R)
        nc.sync.dma_start(w_o_sb[:], w_o[:])

        # per-chunk projection tiles (separate tags so attention on chunk c
        # only depends on projections of chunks <= c)
        qk_t = [
            big.tile([64, 2, 2, QC], F32R, tag=f"qk{n}", name=f"qk{n}")
            for n in range(nqc)
        ]
        v_t = [
            big.tile([128, 4, 130], F32R, tag=f"v{n}", name=f"v{n}")
            for n in range(nqc)
        ]
        for n in range(nqc):
            nc.vector.tensor_copy(
                v_t[n][:].rearrange("p t (g c) -> p t g c", g=2)[:, :, :, 64:65],
                ones_f[:].rearrange("p (t g o) -> p t g o", g=2, o=1),
            )

        for _rep in range(repeat):
            def emit_proj(n):
                xts = []
                for half in range(2):
                    xt = xpool.tile([128, 4, QC], F32R)
                    src = xT.rearrange("(t p) s -> p t s", p=128)
                    for q in range(2):
                        nc.gpsimd.dma_start(
                            xt[:, 2 * q:2 * q + 2, :],
                            src[:, 4 * half + 2 * q:4 * half + 2 * q + 2,
                                n * QC:(n + 1) * QC],
                        )
                    xts.append(xt)
                stage = stpool.tile([128, 2, QC], F32R)
                for m in range(3):
                    ps = psO.tile([128, QC], F32, tag="mm512")
                    for t in range(8):
                        nc.tensor.matmul(
                            ps[:],
                            w_sb[:, t, m * 128:(m + 1) * 128],
                            xts[t // 4][:, t % 4, :],
                            start=(t == 0),
                            stop=(t == 7),
                        )
                    if m < 2:
                        nc.vector.tensor_scalar_add(
                            qk_t[n][:, 0, m, :], ps[0:64, :], biases[0:64, m:m + 1]
                        )
                        nc.vector.tensor_scalar_add(
                            stage[64:128, m, :], ps[64:128, :],
                            biases[64:128, m:m + 1],
                        )
                        if m == 1:
                            nc.gpsimd.dma_start(
                                qk_t[n][:, 1, :, :], stage[64:128, :, :]
                            )
                    else:
                        vt_c = vtpool.tile([128, QC], F32R)
                        nc.vector.tensor_scalar_add(
                            vt_c[:], ps[:], biases[:, 2:3]
                        )
                        tr = psS.tile([128, 4, 128], F32R, tag="sc")
                        for s in range(4):
                            nc.tensor.transpose(
                                tr[:, s, :], vt_c[:, s * 128:(s + 1) * 128], ident[:]
                            )
                        nc.vector.tensor_copy(
                            v_t[n][:].rearrange("p t (g c) -> p t g c", g=2)[:, :, :, 0:64],
                            tr[:].rearrange("p t (g c) -> p t g c", g=2),
                        )

            def emit_jloop(c):
                ctxA = psCA.tile([65, QC], F32, tag="ctxA")
                ctxB = psCB.tile([65, QC], F32, tag="ctxB")
                jmax = 4 * (c + 1)
                for j in range(jmax):
                    p = j - 4 * c
                    off = max(0, p) * KT
                    n_j, s_j = j // 4, j % 4
                    sc = psS.tile([128, 2, QC], F32)
                    for h in range(2):
                        nc.tensor.matmul(
                            sc[:, h, off:],
                            qk_t[n_j][:, h, 1, s_j * KT:(s_j + 1) * KT],
                            qk_t[c][:, h, 0, off:],
                            start=True, stop=True,
                        )
                    at = apool.tile([128, 2, QC], F32R)
                    nc.scalar.activation(
                        at[:, :, off:], sc[:, :, off:], AF.Exp, scale=0.125
                    )
                    if p >= 0:
                        # zero the upper-triangular wedge in place (both heads
                        # in one op; head dim has pattern step 0):
                        # keep iff (off + q_local) - k - 128*p >= 0
                        nc.gpsimd.affine_select(
                            out=at[:, :, off:], in_=at[:, :, off:],
                            pattern=[[0, 2], [1, QC - off]],
                            compare_op=AluOpType.is_ge,
                            fill=0.0, base=off - KT * p, channel_multiplier=-1,
                        )
                    first, last = (j == 0), (j == jmax - 1)
                    nc.tensor.matmul(
                        ctxA[:, off:], v_t[n_j][:, s_j, 0:65], at[:, 0, off:],
                        start=first, stop=last,
                    )
                    nc.tensor.matmul(
                        ctxB[:, off:], v_t[n_j][:, s_j, 65:130], at[:, 1, off:],
                        start=first, stop=last,
                    )
                return ctxA, ctxB
            def emit_norm(c, ctxA, ctxB):
                recip = npool.tile([65, 2, QC], F32, tag="recip")
                nc.vector.reciprocal(recip[64:65, 0, :], ctxA[64:65, :])
                nc.vector.reciprocal(recip[64:65, 1, :], ctxB[64:65, :])
                scr = npool.tile([1, 2, QC], F32, tag="scr")
                nc.gpsimd.dma_start(scr[:], recip[64:65, :, :])
                bc = npool.tile([64, 2, QC], F32, tag="bc")
                nc.gpsimd.partition_broadcast(bc[:], scr[:])
                ctxn = npool.tile([128, QC], F32R, tag="ctxn")
                nc.vector.tensor_mul(ctxn[0:64, :], ctxA[0:64, :], bc[:, 0, :])
                ctxnB = npool.tile([64, QC], F32R, tag="ctxnB")
                nc.vector.tensor_mul(ctxnB[:], ctxB[0:64, :], bc[:, 1, :])
                nc.gpsimd.dma_start(ctxn[64:128, :], ctxnB[:])

                for s in range(4):
                    ot = opool.tile([128, D], F32)
                    for half in range(2):
                        op = psO.tile([128, QC], F32, tag="mm512")
                        nc.tensor.matmul(
                            op[:],
                            ctxn[:, s * 128:(s + 1) * 128],
                            w_o_sb[:, half * QC:(half + 1) * QC],
                            start=True, stop=True,
                        )
                        nc.any.tensor_copy(
                            ot[:, half * QC:(half + 1) * QC], op[:]
                        )
                    row = c * QC + s * 128
                    nc.sync.dma_start(outp[row:row + 128, :], ot[:])


            emit_proj(0)
            for c in range(nqc):
                _ctx = emit_jloop(c)
                if c + 1 < nqc:
                    emit_proj(c + 1)
                emit_norm(c, *_ctx)
    nc.compile()
    return nc


_PROGRAM_CACHE: dict = {}


def _get_program(S: int):
    if S not in _PROGRAM_CACHE:
        _PROGRAM_CACHE[S] = build_program(S)
    return _PROGRAM_CACHE[S]


def make_in_maps(x, w_qkv, b_qkv, w_out):
    x = np.asarray(x, dtype=np.float32)
    w_qkv = np.asarray(w_qkv, dtype=np.float32)
    b_qkv = np.asarray(b_qkv, dtype=np.float32)
    w_out = np.asarray(w_out, dtype=np.float32)
    S = x.shape[1]
    xT = np.ascontiguousarray(x.reshape(S, D).T)
    in_maps = []
    for c in range(N_CORES):
        lo, hi = 128 * c, 128 * (c + 1)
        w_shard = np.ascontiguousarray(
            np.concatenate(
                [w_qkv[:, lo:hi], w_qkv[:, D + lo:D + hi], w_qkv[:, 2 * D + lo:2 * D + hi]],
                axis=1,
            )
        )
        b_shard = np.concatenate(
            [b_qkv[lo:hi], b_qkv[D + lo:D + hi], b_qkv[2 * D + lo:2 * D + hi]]
        )
        w_o_shard = np.ascontiguousarray(w_out[lo:hi, :])
        in_maps.append(
            {"xT": xT, "w_sh": w_shard, "b_sh": b_shard, "w_o": w_o_shard}
        )
    return in_maps


def kernel(x, w_qkv, b_qkv, w_out, b_out):
    x = np.asarray(x, dtype=np.float32)
    b_out = np.asarray(b_out, dtype=np.float32)
    B, S, _ = x.shape
    in_maps = make_in_maps(x, w_qkv, b_qkv, w_out)
    nc = _get_program(S)
    res = run_bass_kernel_spmd(nc, in_maps, list(range(N_CORES))).results
    out = res[0]["outp"].copy()
    for c in range(1, N_CORES):
        out += res[c]["outp"]
    out += b_out
    return out.reshape(B, S, D)
